# revision 42
# baseline (speedup 1.0000x reference)
"""Trainium2 Bass kernel for the constrained leaky RNN.

Model (reference):
    x_proj = inputs @ W_in.T + b_in                    # [B, T, H]
    h_t    = (1-a)*h_{t-1} + a*tanh(x_proj_t + h_{t-1} @ W_rec.T + h_bias)
    out    = hs @ W_out.T + b_out                      # [B, T, O]
with B=64, T=2048, I=128, H=512, O=64, a=0.2.

Strategy (v8):
  - Data-parallel over batch: 8 cores x 8 batch rows, no collectives.
  - State transposed on-chip: H on partitions (4 tiles of 128), batch (8)
    on the free dim.
  - The whole per-step serial chain is reduced to: psum slot closes ->
    tanh (reads PSUM directly) -> recurrent matmuls. Everything else is
    off the critical path:
      slot_{t+1} = xd-proj_{t+1} (+beta) + 0.8*pre_t (identity matmul)
                   + g_t @ (a W_rec).T
      g_t   = tanh(slot_t / S)           [slot_t == pre_t by construction]
      pre_t = fp16 copy of slot_t to SBUF (DVE, off-chain; feeds the
              0.8*I matmul of step t+1)
    where xd_t = x_t - 0.8 x_{t-1} is differenced on the host so that the
    0.8-decay of the input projection telescopes into the identity matmul.
  - Each step's slot is split across TWO psum banks (A: j0,j1 / B: j2,j3)
    because Tile tracks PSUM dependencies at bank granularity: tanh half A
    fires as soon as bank A closes, overlapping the remaining matmuls.
  - h_t is reconstructed off-chain (Hs_t = 0.8 Hs_{t-1} + g_t on DVE) and
    the output projection is batched per 32-step chunk, its matmuls and
    PSUM-evacuation spread into the next chunk's early steps.
"""

import os
import sys

sys.path.insert(0, "/opt/trn_rl_repo")

import numpy as np

B, T, I, H, O = 64, 2048, 128, 512, 64
NCORES = 8
BL = B // NCORES          # batch rows per core
ALPHA = 0.2
DECAY = 1.0 - ALPHA
TC = 32                   # steps per chunk (xc DMA / hs buffer / outproj)
NCHUNK = T // TC

DT_REC = "fp16"           # weights/state dtype on chip
SCALE = 1.0               # scale folded into W_in/W_rec/beta; tanh scale=1/S

_BUILD_CACHE = {}


def _build(dt_flag: str, with_beta: bool):
    import concourse.tile as tile
    from concourse import bacc, mybir
    from contextlib import ExitStack

    f32 = mybir.dt.float32
    dt_rec = {"fp32": f32, "bf16": mybir.dt.bfloat16,
              "fp16": mybir.dt.float16}[dt_flag]
    Alu = mybir.AluOpType
    Act = mybir.ActivationFunctionType

    nc = bacc.Bacc("TRN2")
    xT = nc.dram_tensor("xT", [I, T * BL], dt_rec, kind="ExternalInput")
    wrecT = nc.dram_tensor("wrecT", [H, H], dt_rec, kind="ExternalInput")
    winT = nc.dram_tensor("winT", [I, H], dt_rec, kind="ExternalInput")
    id08 = nc.dram_tensor("id08", [128, 128], dt_rec, kind="ExternalInput")
    beta = nc.dram_tensor("beta", [1, H], dt_rec, kind="ExternalInput")
    woutT = nc.dram_tensor("woutT", [H, O], dt_rec, kind="ExternalInput")
    bout = nc.dram_tensor("bout", [O, 1], f32, kind="ExternalInput")
    outT = nc.dram_tensor("outT", [O, T * BL], f32, kind="ExternalOutput")

    inv_s = 1.0 / SCALE

    with ExitStack() as ctx:
        tc = ctx.enter_context(tile.TileContext(nc))
        const = ctx.enter_context(tc.tile_pool(name="const", bufs=1))
        xpool = ctx.enter_context(tc.tile_pool(name="xpool", bufs=2))
        gpool = ctx.enter_context(tc.tile_pool(name="gpool", bufs=4))
        prepool = ctx.enter_context(tc.tile_pool(name="prepool", bufs=4))
        hspool = ctx.enter_context(tc.tile_pool(name="hspool", bufs=2))
        opool = ctx.enter_context(tc.tile_pool(name="opool", bufs=2))
        xtpool = ctx.enter_context(tc.tile_pool(name="xtpool", bufs=2))
        # full-bank PSUM tiles: 3+3 (slot A/B) + 1 (xp staging) + 1 (outproj)
        psA = ctx.enter_context(tc.tile_pool(name="psA", bufs=3, space="PSUM"))
        psB = ctx.enter_context(tc.tile_pool(name="psB", bufs=3, space="PSUM"))
        ps_xp = ctx.enter_context(tc.tile_pool(name="ps_xp", bufs=1, space="PSUM"))
        ps_o = ctx.enter_context(tc.tile_pool(name="ps_o", bufs=1, space="PSUM"))

        # ---- constants ----
        wrec_sb = const.tile([128, 4, H], dt_rec)       # [:, i, j*128+m]
        for i in range(4):
            nc.sync.dma_start(wrec_sb[:, i], wrecT[i * 128:(i + 1) * 128, :])
        win_sb = const.tile([I, H], dt_rec)
        nc.sync.dma_start(win_sb, winT[:, :])
        id_sb = const.tile([128, 128], dt_rec)
        nc.sync.dma_start(id_sb, id08[:, :])
        wout_sb = const.tile([128, 4, O], dt_rec)
        for j in range(4):
            nc.sync.dma_start(wout_sb[:, j], woutT[j * 128:(j + 1) * 128, :])
        bout_sb = const.tile([O, 1], f32)
        nc.sync.dma_start(bout_sb, bout[:, :])
        if with_beta:
            beta_sb = const.tile([1, H], dt_rec)
            nc.sync.dma_start(beta_sb, beta[:, :])
            bcoef_sb = const.tile([1, 2, BL], dt_rec)
            nc.any.memset(bcoef_sb[:, 0], 1.0)
            nc.any.memset(bcoef_sb[:, 1], ALPHA)

        hs_init = const.tile([128, 4, BL], dt_rec)
        nc.any.memzero(hs_init[:])

        x_tiles = {}
        xt_tiles = {}

        def load_chunk(c):
            if c >= NCHUNK:
                return
            xt = xpool.tile([I, TC * BL], dt_rec, tag="x")
            nc.sync.dma_start(xt, xT[:, c * TC * BL:(c + 1) * TC * BL])
            x_tiles[c] = xt

        def fill_xt(c):
            """Bulk x-diff projection for chunk c -> SBUF [128, TC, 4, BL]."""
            if c >= NCHUNK or with_beta:
                return
            buf = xtpool.tile([128, TC, 4, BL], dt_rec, tag="xt")
            xc = x_tiles[c]
            for j in range(4):
                psx = ps_xp.tile([128, TC * BL], f32, tag="psxp")
                nc.tensor.matmul(psx, win_sb[:, j * 128:(j + 1) * 128],
                                 xc[:, :], start=True, stop=True)
                nc.vector.tensor_scalar_mul(out=buf[:, :, j, :],
                                            in0=psx.rearrange(
                                                "p (t b) -> p t b", b=BL),
                                            scalar1=1.0)
            xt_tiles[c] = buf

        load_chunk(0)
        load_chunk(1)
        fill_xt(0)
        fill_xt(1)

        def slot_view(j, slots):
            """(slot_half, local_j) for output tile j."""
            return slots[j // 2], j % 2

        def xd_beta_matmuls(t, slots):
            """x-diff (+beta) matmuls opening step t's two slot halves."""
            c, tl = divmod(t, TC)
            xc = x_tiles[c]
            for j in range(4):
                sv, lj = slot_view(j, slots)
                nc.tensor.matmul(
                    sv[:, lj],
                    win_sb[:, j * 128:(j + 1) * 128],
                    xc[:, tl * BL:(tl + 1) * BL],
                    start=(lj == 0), stop=False,
                    skip_group_check=True,
                )
            if with_beta:
                sel = 0 if t == 0 else 1
                for j in range(4):
                    sv, lj = slot_view(j, slots)
                    nc.tensor.matmul(
                        sv[:, lj],
                        beta_sb[:, j * 128:(j + 1) * 128],
                        bcoef_sb[:, sel],
                        start=False, stop=False,
                        skip_group_check=True,
                    )

        def new_slots():
            a_full = psA.tile([128, 32, 2, BL], f32, tag="slotA")
            b_full = psB.tile([128, 32, 2, BL], f32, tag="slotB")
            return (a_full[:, 0], b_full[:, 0])

        def outproj(hsc, oc):
            pso = ps_o.tile([O, TC * BL], f32, tag="pso")
            for j in range(4):
                nc.tensor.matmul(pso, wout_sb[:, j], hsc[:, j],
                                 start=(j == 0), stop=(j == 3))
            ob = opool.tile([O, TC * BL], f32, tag="ob")
            nc.scalar.activation(ob[:], pso, Act.Identity,
                                 bias=bout_sb[:, 0:1])
            nc.sync.dma_start(outT[:, oc * TC * BL:(oc + 1) * TC * BL], ob[:])

        # step 0: xp only (h_{-1} = 0)
        slots_t = new_slots()
        if with_beta:
            xd_beta_matmuls(0, slots_t)
        else:
            tmp0 = prepool.tile([128, 4, BL], dt_rec, tag="pre")
            nc.vector.tensor_scalar_mul(out=tmp0[:], in0=xt_tiles[0][:, 0],
                                        scalar1=1.0)
            for half in range(2):
                nc.tensor.matmul(slots_t[half][:, 0:2], id_sb[:, :],
                                 tmp0[:, 2 * half:2 * half + 2],
                                 start=True, stop=True,
                                 skip_group_check=True)
        hs_prev = hs_init[:]
        hs_chunk = None
        prev_hs_chunk = None

        for t in range(T):
            c, tl = divmod(t, TC)
            if tl == 0:
                prev_hs_chunk = hs_chunk
                hs_chunk = hspool.tile([128, 4, TC, BL], dt_rec, tag="hs")
                if t > 0:
                    load_chunk(c + 1)
                    fill_xt(c + 1)

            # g_t = tanh(slot_t / S): half A fires when bank A closes
            g = gpool.tile([128, 4, BL], dt_rec, tag="g")
            nc.scalar.activation(g[:, 0:2], slots_t[0], Act.Tanh, scale=inv_s)
            nc.scalar.activation(g[:, 2:4], slots_t[1], Act.Tanh, scale=inv_s)

            # off-chain, feeds step t+1's identity matmul:
            #   pre_n = 0.8*slot_t + xtilde_{t+1}     (bulk-projected x-diff)
            pre_n = prepool.tile([128, 4, BL], dt_rec, tag="pre")
            if with_beta or t == T - 1:
                nc.vector.tensor_scalar_mul(out=pre_n[:, 0:2],
                                            in0=slots_t[0], scalar1=DECAY)
                nc.vector.tensor_scalar_mul(out=pre_n[:, 2:4],
                                            in0=slots_t[1], scalar1=DECAY)
            else:
                c2, tl2 = divmod(t + 1, TC)
                xtb = xt_tiles[c2]
                nc.vector.scalar_tensor_tensor(
                    out=pre_n[:, 0:2], in0=slots_t[0], scalar=DECAY,
                    in1=xtb[:, tl2, 0:2], op0=Alu.mult, op1=Alu.add,
                )
                nc.vector.scalar_tensor_tensor(
                    out=pre_n[:, 2:4], in0=slots_t[1], scalar=DECAY,
                    in1=xtb[:, tl2, 2:4], op0=Alu.mult, op1=Alu.add,
                )

            # Hs_t = 0.8 * Hs_{t-1} + g_t   (off critical path)
            nc.vector.scalar_tensor_tensor(
                out=hs_chunk[:, :, tl], in0=hs_prev, scalar=DECAY,
                in1=g[:], op0=Alu.mult, op1=Alu.add,
            )
            hs_prev = hs_chunk[:, :, tl]

            if t < T - 1:
                slots_n = new_slots()
                if with_beta:
                    xd_beta_matmuls(t + 1, slots_n)
                # recurrent matmuls += g_t @ (a W_rec).T
                # Bank-A-critical order: everything that closes bank A
                # (j0,j1) is emitted before any bank-B work, so the next
                # step's tanh half A fires ~4 matmuls earlier:
                #   [A:i01, id_A, A:i23] then [B:i01, id_B, B:i23]
                # Per bank: 8 rec matmuls, then ONE N=16 identity matmul
                # (same stationary for both j-regions) as the bank's single
                # final writer - so each tanh carries exactly one wait and
                # no merged EVENT_SEMAPHORE holds tanh_A hostage on bank B.
                for half in range(2):
                    js = (2 * half, 2 * half + 1)
                    for i in range(4):
                        for j in js:
                            sv, lj = slot_view(j, slots_n)
                            nc.tensor.matmul(
                                sv[:, lj],
                                wrec_sb[:, i, j * 128:(j + 1) * 128],
                                g[:, i],
                                start=(not with_beta and i == 0 and lj == 0),
                                stop=False,
                                skip_group_check=True,
                            )
                    nc.tensor.matmul(
                        slots_n[half][:, 0:2], id_sb[:, :],
                        pre_n[:, 2 * half:2 * half + 2],
                        start=False, stop=True,
                        skip_group_check=True,
                    )
                slots_t = slots_n

            # deferred output projection for the previous chunk
            if tl == 2 and prev_hs_chunk is not None:
                outproj(prev_hs_chunk, c - 1)

        outproj(hs_chunk, NCHUNK - 1)

    nc.finalize()
    return nc


def _get_nc(dt_flag: str, with_beta: bool):
    key = (dt_flag, with_beta)
    if key not in _BUILD_CACHE:
        _BUILD_CACHE[key] = _build(dt_flag, with_beta)
    return _BUILD_CACHE[key]


def _prep_in_maps(inputs, dt_flag: str):
    import ml_dtypes

    x = np.asarray(inputs["inputs"], dtype=np.float32)
    W_in = np.asarray(inputs["W_in"], dtype=np.float32)
    b_in = np.asarray(inputs["b_in"], dtype=np.float32)
    W_rec = np.asarray(inputs["W_rec"], dtype=np.float32)
    h_bias = np.asarray(inputs["h_bias"], dtype=np.float32)
    W_out = np.asarray(inputs["W_out"], dtype=np.float32)
    b_out = np.asarray(inputs["b_out"], dtype=np.float32)

    dt = {"fp32": np.float32, "bf16": ml_dtypes.bfloat16,
          "fp16": np.float16}[dt_flag]

    # x differencing: xd_0 = x_0 ; xd_t = x_t - 0.8 x_{t-1}
    xd = x.copy()
    xd[:, 1:] -= DECAY * x[:, :-1]

    wrecT = np.ascontiguousarray((SCALE * ALPHA * W_rec.T).astype(dt))
    winT = np.ascontiguousarray((SCALE * W_in.T).astype(dt))
    id08 = np.ascontiguousarray(np.eye(128).astype(dt))
    beta_v = np.ascontiguousarray(
        (SCALE * (b_in + h_bias)).astype(dt).reshape(1, H))
    with_beta = bool(np.any(np.asarray(beta_v, dtype=np.float32) != 0))
    woutT = np.ascontiguousarray((ALPHA * W_out.T).astype(dt))
    bout = np.ascontiguousarray(b_out.reshape(O, 1))

    in_maps = []
    for c in range(NCORES):
        xc = xd[c * BL:(c + 1) * BL]                    # [BL, T, I]
        xTc = np.ascontiguousarray(
            xc.transpose(2, 1, 0).reshape(I, T * BL).astype(dt))
        in_maps.append({
            "xT": xTc, "wrecT": wrecT, "winT": winT, "id08": id08,
            "beta": beta_v, "woutT": woutT, "bout": bout,
        })
    return in_maps, with_beta


def _run(inputs, trace=False, dt_flag=None, tmpdir=None):
    from concourse import bass_utils

    if dt_flag is None:
        dt_flag = DT_REC
    in_maps, with_beta = _prep_in_maps(inputs, dt_flag)
    nc = _get_nc(dt_flag, with_beta)
    res = bass_utils.run_bass_kernel_spmd(
        nc, in_maps, core_ids=list(range(NCORES)), trace=trace, tmpdir=tmpdir,
    )
    outs = []
    for c in range(NCORES):
        oT = res.results[c]["outT"]                     # [O, T*BL]
        outs.append(oT.reshape(O, T, BL).transpose(2, 1, 0))
    full = np.concatenate(outs, axis=0).astype(np.float32)
    return full, res


def kernel(**inputs) -> np.ndarray:
    out, _ = _run(inputs, trace=False)
    return out
